# revision 1
# baseline (speedup 1.0000x reference)
"""Trainium2 Bass kernel for the BitwiseAutoencoder problem.

Pipeline (per core, data-parallel over batch: 8 of 64 batches per core):
  1. conv1d(1->256, k=256, stride=16, pad=256) as bf16 matmuls against a
     stride-replicated frame matrix R (one gather DMA per batch, resident).
  2. PSUM eviction fuses relu + bias on the Activation engine (multi-bank
     ops), writing H directly as bf16; the eviction's accum_out produces
     sum(h) for free.  sum(h^2) comes from DVE bn_stats on most units plus
     an ACT Square-with-accum on the trailing group of each half.
  3. [Sh, Sh2] all-gathered across the 8 cores; BN affine folded into the
     transposed-conv weights (a*W2, bf16) and a per-phase bias vector.
  4. convT(256->1, k=256, stride=16) as bf16 matmuls; tap-half fold done in
     PSUM via shifted rhs; the remaining 8 tap groups are regrouped through
     a DRAM bounce (one 4-dim gather DMA per tile) and folded with a bf16
     add tree on DVE.  Output is written phase-major (contiguous DMA) and
     transposed on the host.

Self-contained: shapes/sharding hardcoded for x: [64, 1, 32768] f32, 8 cores.
"""

import numpy as np

import concourse.bass as bass
from concourse import bacc, mybir, tile
from concourse.bass_utils import run_bass_kernel_spmd

N_CORES = 8
B_FULL = 64
BPC = B_FULL // N_CORES  # 8 batches per core
T = 32768
K = 256
S = 16
BN_EPS = 1e-5

XP = T + 2 * K  # padded x length per batch (33280)
L = (T + 2 * K - K) // S + 1  # conv output length (2065)
RW = 2073  # R width: l in [0, 2064+8]
PW = XP // S  # 2080 phase columns

UW = 413  # conv matmul unit width (L = 5*413)
# per-cc eviction groups in units: small leading groups cut pipeline lead-in
GROUP_UNITS = [1, 3] + [4] * 9
G_STARTS = [0, 1, 4, 8, 12, 16, 20, 24, 28, 32, 36]
NGC = len(GROUP_UNITS)  # 11 per cc

# deconv output tiles over u' in [16, 2064); last tile split for a short drain
U_TILES = [(16, 683), (699, 683), (1382, 341), (1723, 341)]
OFW = 690  # of2 free width (max wt + 7)

F32 = mybir.dt.float32
BF16 = mybir.dt.bfloat16
AF = mybir.ActivationFunctionType
ALU = mybir.AluOpType


def _flat_ap(tl, n0, dims):
    """Raw AP over an SBUF tile at flat free-offset n0 with given free dims."""
    full = tl[:]
    return bass.AP(tensor=full.tensor, offset=full.offset + n0,
                   ap=[[full.ap[0][0], 128]] + dims)


def _build():
    nc = bacc.Bacc("TRN2", target_bir_lowering=False, debug=False)

    # ---- external I/O ----
    xph_t = nc.dram_tensor("x_ph", [BPC, 16, PW], BF16, kind="ExternalInput")
    w1t_t = nc.dram_tensor("w1t", [128, 2, K], BF16, kind="ExternalInput")
    vecs_t = nc.dram_tensor("vecs", [128, 2, 3], F32, kind="ExternalInput")
    w2_t = nc.dram_tensor("w2", [128, 2, K], F32, kind="ExternalInput")
    w2fold_t = nc.dram_tensor("w2fold", [128, 2, 16], F32, kind="ExternalInput")
    cb16_t = nc.dram_tensor("cb16", [16], F32, kind="ExternalInput")
    y_t = nc.dram_tensor("y", [BPC, 16, 2048], F32, kind="ExternalOutput")

    with tile.TileContext(nc) as tc:
        with (
            tc.tile_pool(name="persist", bufs=1) as persist,
            tc.tile_pool(name="sqpool", bufs=1) as sqpool,
            tc.tile_pool(name="of2pool", bufs=2) as of2pool,
            tc.tile_pool(name="t4pool", bufs=2) as t4pool,
            tc.tile_pool(name="etpool", bufs=2) as etpool,
            tc.tile_pool(name="yacc", bufs=2) as yaccpool,
            tc.tile_pool(name="smalls", bufs=1) as smalls,
            tc.tile_pool(name="dram", bufs=1, space="DRAM") as dram,
        ):
            # R frame matrices, one per batch, all resident.  Even batches go
            # through SWDGE (Pool) so the first loads overlap the HWDGE queue.
            R = [persist.tile([128, RW], BF16, tag=f"R{b}", name=f"R{b}")
                 for b in range(BPC)]

            def load_r(b):
                eng = nc.gpsimd if b % 2 == 0 else nc.sync
                eng.dma_start(
                    out=R[b][:],
                    in_=bass.AP(tensor=xph_t, offset=b * XP,
                                ap=[[1, 8], [PW, 16], [1, RW]]),
                )

            load_r(0)
            w1t_sb = persist.tile([128, 2, K], BF16, tag="w1t")
            nc.sync.dma_start(out=w1t_sb[:], in_=w1t_t[:, :, :])
            vecs_sb = persist.tile([128, 2, 3], F32, tag="vecs")
            nc.sync.dma_start(out=vecs_sb[:], in_=vecs_t[:, :, :])
            for b in (2, 1, 4, 3, 6, 5):
                load_r(b)
            w2_sb = persist.tile([128, 2, K], F32, tag="w2")
            nc.sync.dma_start(out=w2_sb[:], in_=w2_t[:, :, :])
            load_r(7)
            w2fold_sb = persist.tile([128, 2, 16], F32, tag="w2fold")
            nc.sync.dma_start(out=w2fold_sb[:], in_=w2fold_t[:, :, :])
            cb_sb = persist.tile([16, 1], F32, tag="cb")
            nc.sync.dma_start(out=cb_sb[:], in_=cb16_t[:])
            eps_sb = persist.tile([128, 1], F32, tag="eps")
            nc.vector.memset(eps_sb[:], BN_EPS)
            junk = smalls.tile([128, 1], F32, tag="junk")
            # preload the Relu/Square activation table set while R loads run
            nc.scalar.activation(out=junk[:], in_=eps_sb[:], func=AF.Relu)

            # H: conv output (post-relu) bf16, flat layout (cc, b, l)
            H = persist.tile([128, 2, BPC, L], BF16, tag="H", name="H")
            sums1 = persist.tile([128, 2, NGC], F32, tag="s1", name="s1")
            sums2a = persist.tile([128, 2], F32, tag="s2a", name="s2a")
            stats = persist.tile([128, 80, 6], F32, tag="st", name="st")
            sq = sqpool.tile([128, 4 * UW], BF16, tag="sq", name="sq")

            # ================= phase 1: conv + stats =================
            with tc.tile_pool(name="psum_conv", bufs=2, space="PSUM") as pconv:
                for cc in range(2):
                    for gq in range(NGC):
                        nu_g = GROUP_UNITS[gq]
                        u0 = G_STARTS[gq]
                        ps = pconv.tile([128, 4, 512], F32, tag="pc")
                        for i in range(nu_g):
                            w = u0 + i  # within-cc unit: 5*b + gi
                            b, gi = w // 5, w % 5
                            l0 = UW * gi
                            for h in range(2):
                                nc.tensor.matmul(
                                    ps[:, i, 0:UW],
                                    w1t_sb[:, h, 128 * cc:128 * (cc + 1)],
                                    R[b][:, l0 + 8 * h:l0 + 8 * h + UW],
                                    start=(h == 0), stop=(h == 1),
                                )
                        n0 = UW * u0 + 16520 * cc
                        out_ap = _flat_ap(H, n0, [[UW, nu_g], [1, UW]])
                        # relu+bias eviction -> bf16 H; accum gives sum(h)
                        nc.scalar.activation(
                            out=out_ap, in_=ps[:, 0:nu_g, 0:UW], func=AF.Relu,
                            bias=vecs_sb[:, cc, 0:1], scale=1.0,
                            accum_out=sums1[:, cc, gq:gq + 1],
                        )
                        if gq == NGC - 1:
                            # trailing group's sum(h^2) on ACT
                            nc.scalar.activation(
                                out=sq[:, 0:nu_g * UW],
                                in_=_flat_ap(H, n0, [[1, nu_g * UW]]),
                                func=AF.Square,
                                accum_out=sums2a[:, cc:cc + 1],
                            )
                        else:
                            for i in range(nu_g):
                                u = 40 * cc + u0 + i
                                nc.vector.bn_stats(
                                    out=stats[:, u, :],
                                    in_=_flat_ap(H, 413 * (u0 + i) + 16520 * cc,
                                                 [[1, UW]]),
                                )
            # preload the Sqrt/Copy table set during the collective window
            nc.scalar.activation(out=junk[:], in_=eps_sb[:], func=AF.Sqrt,
                                 bias=eps_sb[:, 0:1])

            # ================= phase 2: global BN =================
            bounce_in = dram.tile([128, 4], F32)
            bounce_out = dram.tile([N_CORES, 128, 4], F32)
            pk = smalls.tile([128, 4], F32, tag="pk")
            n_rest = float(36 * UW)
            for cc in range(2):
                # total sum(h) for this half
                nc.vector.reduce_sum(pk[:, 2 * cc:2 * cc + 1],
                                     sums1[:, cc, :],
                                     axis=mybir.AxisListType.X)
                # sum(h^2): ACT trailing group + bn_stats units (first 36)
                mv = smalls.tile([128, 2], F32, tag=f"mv{cc}", name=f"mv{cc}")
                nc.vector.bn_aggr(out=mv[:],
                                  in_=stats[:, 40 * cc:40 * cc + 36, :])
                e2r = smalls.tile([128, 1], F32, tag=f"e2r{cc}", name=f"e2r{cc}")
                nc.vector.tensor_mul(e2r[:], mv[:, 0:1], mv[:, 0:1])
                nc.vector.tensor_add(e2r[:], e2r[:], mv[:, 1:2])
                nc.vector.tensor_scalar_mul(e2r[:], e2r[:], n_rest)
                nc.vector.tensor_add(pk[:, 2 * cc + 1:2 * cc + 2],
                                     e2r[:], sums2a[:, cc:cc + 1])
            nc.sync.dma_start(out=bounce_in[:, :], in_=pk[:])
            nc.gpsimd.collective_compute(
                "AllGather",
                mybir.AluOpType.bypass,
                replica_groups=[list(range(N_CORES))],
                ins=[bounce_in.opt()],
                outs=[bounce_out.opt()],
            )
            gall = smalls.tile([128, 4, N_CORES], F32, tag="gall")
            nc.sync.dma_start(
                out=gall[:],
                in_=bass.AP(tensor=bounce_out.tensor, offset=bounce_out.offset,
                            ap=[[4, 128], [1, 4], [512, N_CORES]]),
            )
            gsum = smalls.tile([128, 4], F32, tag="gsum")
            nc.vector.reduce_sum(gsum[:], gall[:], axis=mybir.AxisListType.X)
            inv_n = 1.0 / (N_CORES * BPC * L)
            # fold BN scale into deconv weights -> bf16 (kc = 0 first so the
            # first deconv matmuls can start as early as possible)
            w2a = persist.tile([128, 2, K], BF16, tag="w2a", name="w2a")
            a_sb, d_sb = [], []
            for cc in range(2):
                mE = smalls.tile([128, 2], F32, tag=f"mE{cc}", name=f"mE{cc}")
                nc.vector.tensor_scalar_mul(mE[:], gsum[:, 2 * cc:2 * cc + 2], inv_n)
                gvar = smalls.tile([128, 1], F32, tag=f"gv{cc}", name=f"gv{cc}")
                nc.vector.tensor_mul(gvar[:], mE[:, 0:1], mE[:, 0:1])
                nc.vector.tensor_sub(gvar[:], mE[:, 1:2], gvar[:])
                sd = smalls.tile([128, 1], F32, tag=f"sd{cc}", name=f"sd{cc}")
                nc.scalar.activation(out=sd[:], in_=gvar[:], func=AF.Sqrt,
                                     bias=eps_sb[:, 0:1], scale=1.0)
                rinv = smalls.tile([128, 1], F32, tag=f"ri{cc}", name=f"ri{cc}")
                nc.vector.reciprocal(rinv[:], sd[:])
                a = smalls.tile([128, 1], F32, tag=f"a{cc}", name=f"a{cc}")
                nc.vector.tensor_mul(a[:], rinv[:], vecs_sb[:, cc, 1:2])
                nc.vector.tensor_scalar_mul(w2_sb[:, cc, :], w2_sb[:, cc, :],
                                            a[:, 0:1])
                nc.vector.tensor_copy(w2a[:, cc, :], w2_sb[:, cc, :])
                d = smalls.tile([128, 1], F32, tag=f"d{cc}", name=f"d{cc}")
                nc.vector.tensor_mul(d[:], a[:], mE[:, 0:1])
                nc.vector.tensor_sub(d[:], vecs_sb[:, cc, 2:3], d[:])
                a_sb.append(a)
                d_sb.append(d)

            with (
                tc.tile_pool(name="psum_cp", bufs=1, space="PSUM") as psum_cp,
                tc.tile_pool(name="psum_dec", bufs=4, space="PSUM") as pdec,
            ):
                # per-phase bias: cp[p] = sum_k w2fold[k, p] d[k] + ct_scale*ct_b
                pcp = psum_cp.tile([16, 1], F32, tag="pcp")
                nc.tensor.matmul(pcp[:], w2fold_sb[:, 0, :], d_sb[0][:],
                                 start=True, stop=False)
                nc.tensor.matmul(pcp[:], w2fold_sb[:, 1, :], d_sb[1][:],
                                 start=False, stop=True)
                cp16 = smalls.tile([16, 1], F32, tag="cp16")
                nc.vector.tensor_add(cp16[:], pcp[:], cb_sb[:])
                cp_dram = dram.tile([16], F32)
                nc.sync.dma_start(out=cp_dram[:], in_=cp16[:])
                # cpb[8p + b] = cp[p]
                cpb = smalls.tile([128, 1], F32, tag="cpb")
                nc.sync.dma_start(
                    out=cpb[:],
                    in_=bass.AP(tensor=cp_dram.tensor, offset=cp_dram.offset,
                                ap=[[1, 16], [0, 8], [0, 1]]),
                )

                # ================= phase 3: deconv =================
                of2d = [dram.tile([128, BPC, OFW], BF16, name=f"of2d{i}")
                        for i in range(2)]
                for ti, (w0, wt) in enumerate(U_TILES):
                    w7 = wt + 7
                    dbuf = of2d[ti % 2]
                    for b in range(BPC):
                        if b % 2 == 0:
                            of2 = of2pool.tile([128, 2, OFW], BF16, tag="OF2",
                                               name=f"of2_{w0}_{b}")
                        # tap-half fold in PSUM: OF2[r, n] = OF[r, n] + OF[r+128, n-8]
                        for si, s0 in enumerate(range(0, w7, 504)):
                            sw = min(504, w7 - s0)
                            ps = pdec.tile([128, 504], F32, tag="pd")
                            nmm = 0
                            for kc in range(2):
                                for th, off in ((0, 7), (128, 15)):
                                    nc.tensor.matmul(
                                        ps[:, :sw],
                                        w2a[:, kc, th:th + 128],
                                        H[:, kc, b, w0 - off + s0:w0 - off + s0 + sw],
                                        start=(nmm == 0), stop=(nmm == 3),
                                    )
                                    nmm += 1
                            # evictions split between ACT and DVE
                            dst = of2[:, b % 2, s0:s0 + sw]
                            if si == 0 and wt > 341:
                                nc.scalar.copy(dst, ps[:, :sw])
                            elif wt > 341:
                                nc.vector.tensor_copy(dst, ps[:, :sw])
                            elif b % 2 == 0:
                                nc.scalar.copy(dst, ps[:, :sw])
                            else:
                                nc.vector.tensor_copy(dst, ps[:, :sw])
                        if b % 2 == 1:
                            # bounce two batches of taps to DRAM (SWDGE)
                            nc.gpsimd.dma_start(
                                out=dbuf[:, b - 1:b + 1, 0:w7],
                                in_=of2[:, :, 0:w7])
                    # partition regroup via one 4-dim gather:
                    # t4[8p + b, m, u] = of2[16m + p, b, u + 7 - m]
                    t4 = t4pool.tile([128, 8, 683], BF16, tag="T4",
                                     name=f"t4_{w0}")
                    nc.sync.dma_start(
                        out=t4[:, :, 0:wt],
                        in_=bass.AP(
                            tensor=dbuf.tensor, offset=dbuf.offset + 7,
                            ap=[[BPC * OFW, 16], [OFW, 8],
                                [16 * BPC * OFW - 1, 8], [1, wt]]),
                    )
                    # fold the 8 tap groups: bf16 add tree, f32 root
                    et = [etpool.tile([128, 683], BF16, tag=f"e{i}",
                                      name=f"e{i}_{w0}") for i in range(6)]
                    for i in range(4):
                        nc.vector.tensor_add(et[i][:, :wt], t4[:, 2 * i, :wt],
                                             t4[:, 2 * i + 1, :wt])
                    nc.vector.tensor_add(et[4][:, :wt], et[0][:, :wt], et[1][:, :wt])
                    nc.vector.tensor_add(et[5][:, :wt], et[2][:, :wt], et[3][:, :wt])
                    ya = yaccpool.tile([128, 683], F32, tag="ya", name=f"ya_{w0}")
                    nc.vector.tensor_add(ya[:, :wt], et[4][:, :wt], et[5][:, :wt])
                    nc.vector.tensor_scalar_add(ya[:, :wt], ya[:, :wt], cpb[:, 0:1])
                    nc.sync.dma_start(
                        out=bass.AP(tensor=y_t, offset=(w0 - 16),
                                    ap=[[2048, 16], [16 * 2048, 8], [1, wt]]),
                        in_=ya[:, :wt],
                    )
    nc.compile()
    return nc


_NC_CACHE = None


def _get_nc():
    global _NC_CACHE
    if _NC_CACHE is None:
        _NC_CACHE = _build()
    return _NC_CACHE


def _host_prep(inputs):
    import ml_dtypes

    conv_w = np.asarray(inputs["conv_w"], dtype=np.float32)
    conv_b = np.asarray(inputs["conv_b"], dtype=np.float32)
    conv_gate = np.asarray(inputs["conv_gate"], dtype=np.float32)
    conv_scale = np.asarray(inputs["conv_scale"], dtype=np.float32)
    bn_gamma = np.asarray(inputs["bn_gamma"], dtype=np.float32)
    bn_beta = np.asarray(inputs["bn_beta"], dtype=np.float32)
    ct_w = np.asarray(inputs["ct_w"], dtype=np.float32)
    ct_b = np.asarray(inputs["ct_b"], dtype=np.float32)
    ct_gate = np.asarray(inputs["ct_gate"], dtype=np.float32)
    ct_scale = np.asarray(inputs["ct_scale"], dtype=np.float32)

    W1 = conv_w[:, 0, :] * (conv_gate[:, 0, :] + 1.0) * 0.5  # [c, j]
    W1 = W1 * conv_scale[:, None]
    bias1 = conv_scale * conv_b
    # w1t[j0, h, c] = W1[c, j0 + 128h]
    w1t = np.ascontiguousarray(
        W1.T.reshape(2, 128, K).transpose(1, 0, 2)).astype(ml_dtypes.bfloat16)

    vecs = np.stack([bias1, bn_gamma, bn_beta], axis=1)  # [256, 3]
    vecs = np.ascontiguousarray(vecs.reshape(2, 128, 3).transpose(1, 0, 2))

    W2 = ct_w[:, 0, :] * (ct_gate[:, 0, :] + 1.0) * 0.5  # [k, j]
    W2 = W2 * float(ct_scale[0])
    w2 = np.ascontiguousarray(W2.reshape(2, 128, K).transpose(1, 0, 2))
    w2fold = W2.reshape(K, 16, 16).sum(axis=1)  # [k, p]
    w2fold = np.ascontiguousarray(w2fold.reshape(2, 128, 16).transpose(1, 0, 2))
    cb16 = np.full(16, float(ct_scale[0]) * float(ct_b[0]), dtype=np.float32)

    return {
        "w1t": w1t,
        "vecs": vecs.astype(np.float32),
        "w2": w2.astype(np.float32),
        "w2fold": w2fold.astype(np.float32),
        "cb16": cb16,
    }


def kernel(**inputs) -> np.ndarray:
    import ml_dtypes

    x = np.asarray(inputs["x"], dtype=np.float32)  # [64, 1, 32768]
    shared = _host_prep(inputs)
    nc = _get_nc()

    in_maps = []
    for c in range(N_CORES):
        shard = x[BPC * c:BPC * (c + 1), 0, :]  # [8, T]
        xpad = np.zeros((BPC, XP), dtype=np.float32)
        xpad[:, K:K + T] = shard
        # phase layout: x_ph[b, p, n] = x_pad[b, 16n + p]
        xph = np.ascontiguousarray(
            xpad.reshape(BPC, PW, 16).transpose(0, 2, 1)).astype(ml_dtypes.bfloat16)
        m = dict(shared)
        m["x_ph"] = xph
        in_maps.append(m)

    res = run_bass_kernel_spmd(nc, in_maps, core_ids=list(range(N_CORES)))
    outs = []
    for c in range(N_CORES):
        yph = res.results[c]["y"].reshape(BPC, 16, 2048)  # [b, p, u]
        outs.append(yph.transpose(0, 2, 1).reshape(BPC, 1, T))
    return np.concatenate(outs, axis=0).astype(np.float32)



# revision 3
# speedup vs baseline: 1.0351x; 1.0351x over previous
"""Trainium2 Bass kernel for the BitwiseAutoencoder problem.

Pipeline (per core, data-parallel over batch: 8 of 64 batches per core):
  1. conv1d(1->256, k=256, stride=16, pad=256) as bf16 matmuls against a
     stride-replicated frame matrix R (one gather DMA per batch, resident).
     PSUM eviction (relu+bias, accum_out -> sum(h)) rotates over the
     Activation / DVE / Pool engines so the conv stays PE-paced; sum(h^2)
     per group via ACT Square-with-accum or DVE square+sum.
  2. [Sh, Sh2] all-gathered across the 8 cores; BN affine folded into the
     transposed-conv weights (a*W2, bf16) and a per-phase bias vector.
     fp32 filler matmuls keep the PE clock ramped through the collective.
  3. convT(256->1, k=256, stride=16) as bf16 matmuls; tap-half fold in PSUM
     via shifted rhs; (m, m+4) tap-group pairs folded on-chip with one
     64-partition DVE add (halves the DRAM bounce); remaining 4 groups are
     regrouped through a DRAM bounce + shifted gather and folded with a
     short add tree.  Output written phase-major, transposed on the host.

Self-contained: shapes/sharding hardcoded for x: [64, 1, 32768] f32, 8 cores.
"""

import numpy as np

import concourse.bass as bass
from concourse import bacc, mybir, tile
from concourse.bass_utils import run_bass_kernel_spmd

N_CORES = 8
B_FULL = 64
BPC = B_FULL // N_CORES  # 8 batches per core
T = 32768
K = 256
S = 16
BN_EPS = 1e-5

XP = T + 2 * K  # padded x length per batch (33280)
L = (T + 2 * K - K) // S + 1  # conv output length (2065)
RW = 2073  # R width: l in [0, 2064+8]
PW = XP // S  # 2080 phase columns

UW = 413  # conv matmul unit width (L = 5*413)
# conv eviction groups in units (per cc): small lead-ins cut pipeline lead-in
CONV_GROUPS = [1, 3] + [3] * 12  # 14 groups, 40 units per cc
CG_STARTS = [sum(CONV_GROUPS[:i]) for i in range(len(CONV_GROUPS))]
NGC = len(CONV_GROUPS)

# deconv output tiles over u' in [16, 2064)
U_TILES = [(16, 342), (358, 342), (700, 342), (1042, 342), (1384, 342),
           (1726, 338)]
W3A = 345  # allocated E width (wt + 3 max)
OFW2 = 349  # of2 free width (wt + 7 max)

DUMMY_N = 21  # fp32 filler matmuls covering the collective window

F32 = mybir.dt.float32
BF16 = mybir.dt.bfloat16
AF = mybir.ActivationFunctionType
ALU = mybir.AluOpType


def _flat_ap(tl, n0, dims):
    """Raw AP over an SBUF tile at flat free-offset n0 with given free dims."""
    full = tl[:]
    return bass.AP(tensor=full.tensor, offset=full.offset + n0,
                   ap=[[full.ap[0][0], 128]] + dims)


def _build():
    nc = bacc.Bacc("TRN2", target_bir_lowering=False, debug=False)

    # ---- external I/O ----
    xph_t = nc.dram_tensor("x_ph", [BPC, 16, PW], BF16, kind="ExternalInput")
    w1t_t = nc.dram_tensor("w1t", [128, 2, K], BF16, kind="ExternalInput")
    vecs_t = nc.dram_tensor("vecs", [128, 2, 3], F32, kind="ExternalInput")
    w2_t = nc.dram_tensor("w2", [128, 2, K], F32, kind="ExternalInput")
    w2fold_t = nc.dram_tensor("w2fold", [128, 2, 16], F32, kind="ExternalInput")
    cb16_t = nc.dram_tensor("cb16", [16], F32, kind="ExternalInput")
    y_t = nc.dram_tensor("y", [BPC, 16, 2048], F32, kind="ExternalOutput")

    with tile.TileContext(nc) as tc:
        with (
            tc.tile_pool(name="persist", bufs=1) as persist,
            tc.tile_pool(name="sqpool", bufs=2) as sqpool,
            tc.tile_pool(name="of2pool", bufs=2) as of2pool,
            tc.tile_pool(name="epool", bufs=2) as epool,
            tc.tile_pool(name="t4pool", bufs=2) as t4pool,
            tc.tile_pool(name="qpool", bufs=2) as qpool,
            tc.tile_pool(name="smalls", bufs=1) as smalls,
            tc.tile_pool(name="dram", bufs=1, space="DRAM") as dram,
            tc.tile_pool(name="pjunk", bufs=1, space="PSUM") as pjunk,
        ):
            # ---- setup loads: weights + R frame matrices (all HWDGE) ----
            w1t_sb = persist.tile([128, 2, K], BF16, tag="w1t")
            nc.sync.dma_start(out=w1t_sb[:], in_=w1t_t[:, :, :])
            vecs_sb = persist.tile([128, 2, 3], F32, tag="vecs")
            nc.scalar.dma_start(out=vecs_sb[:], in_=vecs_t[:, :, :])

            R = [persist.tile([128, RW], BF16, tag=f"R{b}", name=f"R{b}")
                 for b in range(BPC)]
            for b in range(BPC):
                nc.sync.dma_start(
                    out=R[b][:],
                    in_=bass.AP(tensor=xph_t, offset=b * XP,
                                ap=[[1, 8], [PW, 16], [1, RW]]),
                )

            eps_sb = persist.tile([128, 1], F32, tag="eps")
            nc.vector.memset(eps_sb[:], BN_EPS)
            junkf = persist.tile([128, 512], F32, tag="junkf")
            nc.vector.memset(junkf[:], 0.125)
            junk = smalls.tile([128, 1], F32, tag="junk")
            # preload the Relu/Square activation table set while R loads run
            nc.scalar.activation(out=junk[:], in_=eps_sb[:], func=AF.Relu)

            # H: conv output (post-relu) bf16, flat layout (cc, b, l)
            H = persist.tile([128, 2, BPC, L], BF16, tag="H", name="H")
            sums1 = persist.tile([128, 2, NGC], F32, tag="s1", name="s1")
            sums2 = persist.tile([128, 2, NGC], F32, tag="s2", name="s2")
            tsjunk = persist.tile([128, 3 * UW], BF16, tag="tsj", name="tsj")
            junkps = pjunk.tile([128, 512], F32, tag="jp")

            # ================= phase 1: conv + stats =================
            # eviction engine rotation: Pool / ACT / DVE by global group idx
            ev_engines = [nc.gpsimd, nc.scalar, nc.vector]
            with tc.tile_pool(name="psum_conv", bufs=2, space="PSUM") as pconv:
                for cc in range(2):
                    for gq in range(NGC):
                        gidx = cc * NGC + gq
                        nu_g = CONV_GROUPS[gq]
                        u0 = CG_STARTS[gq]
                        ps = pconv.tile([128, 3, 512], F32, tag="pc")
                        for i in range(nu_g):
                            w = u0 + i  # within-cc unit: 5*b + gi
                            b, gi = w // 5, w % 5
                            l0 = UW * gi
                            for h in range(2):
                                nc.tensor.matmul(
                                    ps[:, i, 0:UW],
                                    w1t_sb[:, h, 128 * cc:128 * (cc + 1)],
                                    R[b][:, l0 + 8 * h:l0 + 8 * h + UW],
                                    start=(h == 0), stop=(h == 1),
                                )
                        n0 = UW * u0 + 16520 * cc
                        out_ap = _flat_ap(H, n0, [[UW, nu_g], [1, UW]])
                        eng = ev_engines[gidx % 3]
                        if eng is nc.scalar:
                            nc.scalar.activation(
                                out=out_ap, in_=ps[:, 0:nu_g, 0:UW],
                                func=AF.Relu, bias=vecs_sb[:, cc, 0:1],
                                scale=1.0,
                                accum_out=sums1[:, cc, gq:gq + 1],
                            )
                        else:
                            # (psum + bias) max 0 -> bf16 H; accum -> sum(h)
                            eng.tensor_scalar(
                                out_ap, ps[:, 0:nu_g, 0:UW],
                                vecs_sb[:, cc, 0:1], 0.0,
                                ALU.add, ALU.max,
                                accum_out=sums1[:, cc, gq:gq + 1],
                            )
                        # sum(h^2) for this group
                        ncols = nu_g * UW
                        h_in = _flat_ap(H, n0, [[1, ncols]])
                        if gidx % 7 in (1, 3, 5):
                            sq = sqpool.tile([128, 3 * UW], BF16, tag="sq",
                                             name=f"sq{gidx}")
                            nc.scalar.activation(
                                out=sq[:, 0:ncols], in_=h_in, func=AF.Square,
                                accum_out=sums2[:, cc, gq:gq + 1],
                            )
                        else:
                            sq = sqpool.tile([128, 3 * UW], BF16, tag="sq",
                                             name=f"sq{gidx}")
                            nc.vector.tensor_tensor(sq[:, 0:ncols], h_in,
                                                    h_in, ALU.mult)
                            nc.vector.tensor_scalar(
                                tsjunk[:, 0:ncols], sq[:, 0:ncols], 0.0, None,
                                ALU.add,
                                accum_out=sums2[:, cc, gq:gq + 1],
                            )

            # stats -> pk = [Sh0, Sh2_0, Sh1, Sh2_1]
            pk = smalls.tile([128, 4], F32, tag="pk")
            for cc in range(2):
                nc.vector.reduce_sum(pk[:, 2 * cc:2 * cc + 1],
                                     sums1[:, cc, :],
                                     axis=mybir.AxisListType.X)
                nc.vector.reduce_sum(pk[:, 2 * cc + 1:2 * cc + 2],
                                     sums2[:, cc, :],
                                     axis=mybir.AxisListType.X)
            bounce_in = dram.tile([128, 4], F32)
            bounce_out = dram.tile([N_CORES, 128, 4], F32)
            nc.sync.dma_start(out=bounce_in[:, :], in_=pk[:])

            # remaining weight loads + Sqrt/Copy table preload (ACT queue
            # drains phase-1 work first; ready well before the fold needs them)
            w2_sb = persist.tile([128, 2, K], F32, tag="w2")
            nc.scalar.dma_start(out=w2_sb[:], in_=w2_t[:, :, :])
            w2fold_sb = persist.tile([128, 2, 16], F32, tag="w2fold")
            nc.scalar.dma_start(out=w2fold_sb[:], in_=w2fold_t[:, :, :])
            cb_sb = persist.tile([16, 1], F32, tag="cb")
            nc.scalar.dma_start(out=cb_sb[:], in_=cb16_t[:])
            nc.scalar.activation(out=junk[:], in_=eps_sb[:], func=AF.Sqrt,
                                 bias=eps_sb[:, 0:1])

            # ================= phase 2: global BN =================
            nc.gpsimd.collective_compute(
                "AllGather",
                mybir.AluOpType.bypass,
                replica_groups=[list(range(N_CORES))],
                ins=[bounce_in.opt()],
                outs=[bounce_out.opt()],
            )
            # fp32 fillers keep the PE clock ramped through the collective
            for _ in range(DUMMY_N):
                nc.tensor.matmul(junkps[:], junkf[:, 0:128], junkf[:],
                                 start=True, stop=True)

            gall = smalls.tile([128, 4, N_CORES], F32, tag="gall")
            nc.sync.dma_start(
                out=gall[:],
                in_=bass.AP(tensor=bounce_out.tensor, offset=bounce_out.offset,
                            ap=[[4, 128], [1, 4], [512, N_CORES]]),
            )
            gsum = smalls.tile([128, 4], F32, tag="gsum")
            nc.vector.reduce_sum(gsum[:], gall[:], axis=mybir.AxisListType.X)
            inv_n = 1.0 / (N_CORES * BPC * L)
            # fold BN scale into deconv weights -> bf16 (both cc vectorized;
            # cc = 0 first so the first deconv matmuls start earliest)
            mE = smalls.tile([128, 4], F32, tag="mE")
            nc.vector.tensor_scalar_mul(mE[:], gsum[:], inv_n)
            m2 = smalls.tile([128, 4], F32, tag="m2")
            nc.vector.tensor_mul(m2[:], mE[:], mE[:])
            mE_f = mE[:]
            m2_f = m2[:]
            mE_odd = bass.AP(tensor=mE_f.tensor, offset=mE_f.offset + 1,
                             ap=[[mE_f.ap[0][0], 128], [2, 2]])
            m2_even = bass.AP(tensor=m2_f.tensor, offset=m2_f.offset,
                              ap=[[m2_f.ap[0][0], 128], [2, 2]])
            mE_even = bass.AP(tensor=mE_f.tensor, offset=mE_f.offset,
                              ap=[[mE_f.ap[0][0], 128], [2, 2]])
            gv = smalls.tile([128, 2], F32, tag="gv")
            nc.vector.tensor_sub(gv[:], mE_odd, m2_even)
            sd = smalls.tile([128, 2], F32, tag="sd")
            nc.scalar.activation(out=sd[:], in_=gv[:], func=AF.Sqrt,
                                 bias=eps_sb[:, 0:1], scale=1.0)
            rinv = smalls.tile([128, 2], F32, tag="rinv")
            nc.vector.reciprocal(rinv[:], sd[:])
            vecs_f = vecs_sb[:]
            gamma = bass.AP(tensor=vecs_f.tensor, offset=vecs_f.offset + 1,
                            ap=[[vecs_f.ap[0][0], 128], [3, 2]])
            beta = bass.AP(tensor=vecs_f.tensor, offset=vecs_f.offset + 2,
                           ap=[[vecs_f.ap[0][0], 128], [3, 2]])
            aa = smalls.tile([128, 2], F32, tag="aa")
            nc.vector.tensor_mul(aa[:], rinv[:], gamma)
            w2a = persist.tile([128, 2, K], BF16, tag="w2a", name="w2a")
            for cc in range(2):
                nc.vector.tensor_scalar_mul(w2_sb[:, cc, :], w2_sb[:, cc, :],
                                            aa[:, cc:cc + 1])
                nc.vector.tensor_copy(w2a[:, cc, :], w2_sb[:, cc, :])
            admu = smalls.tile([128, 2], F32, tag="admu")
            nc.vector.tensor_mul(admu[:], aa[:], mE_even)
            dd = smalls.tile([128, 2], F32, tag="dd")
            nc.vector.tensor_sub(dd[:], beta, admu[:])

            with (
                tc.tile_pool(name="psum_cp", bufs=1, space="PSUM") as psum_cp,
                tc.tile_pool(name="psum_dec", bufs=6, space="PSUM") as pdec,
            ):
                pcp = psum_cp.tile([16, 1], F32, tag="pcp")
                cp16 = smalls.tile([16, 1], F32, tag="cp16")
                cp_dram = dram.tile([16], F32)
                cpb = smalls.tile([128, 1], F32, tag="cpb")

                # ================= phase 3: deconv =================
                dbufs = [dram.tile([64, BPC, W3A], BF16, name=f"dbuf{i}")
                         for i in range(2)]
                dec_ev = [nc.vector, nc.scalar, nc.gpsimd]
                for ti, (w0, wt) in enumerate(U_TILES):
                    w7 = wt + 7
                    w3 = wt + 3
                    dbuf = dbufs[ti % 2]
                    E = epool.tile([64, BPC, W3A], BF16, tag="E",
                                   name=f"E_{w0}")
                    for b in range(BPC):
                        if b % 2 == 0:
                            of2 = of2pool.tile([128, 2, OFW2], BF16, tag="OF2",
                                               name=f"of2_{w0}_{b}")
                        # tap-half fold in PSUM via shifted rhs
                        ps = pdec.tile([128, OFW2], F32, tag="pd")
                        nmm = 0
                        for kc in range(2):
                            for th, off in ((0, 7), (128, 15)):
                                nc.tensor.matmul(
                                    ps[:, 0:w7],
                                    w2a[:, kc, th:th + 128],
                                    H[:, kc, b, w0 - off:w0 - off + w7],
                                    start=(nmm == 0), stop=(nmm == 3),
                                )
                                nmm += 1
                        # per-phase bias matmul squeezed in right after the
                        # first deconv tile's first batch (needs dd)
                        if ti == 0 and b == 0:
                            nc.tensor.matmul(pcp[:], w2fold_sb[:, 0, :],
                                             dd[:, 0:1], start=True,
                                             stop=False)
                            nc.tensor.matmul(pcp[:], w2fold_sb[:, 1, :],
                                             dd[:, 1:2], start=False,
                                             stop=True)
                            nc.vector.tensor_add(cp16[:], pcp[:], cb_sb[:])
                            nc.sync.dma_start(out=cp_dram[:], in_=cp16[:])
                            # cpb[8p + b] = cp[p]
                            nc.sync.dma_start(
                                out=cpb[:],
                                in_=bass.AP(tensor=cp_dram.tensor,
                                            offset=cp_dram.offset,
                                            ap=[[1, 16], [0, 8], [0, 1]]),
                            )
                        # eviction rotates DVE / ACT / Pool
                        eng = dec_ev[b % 3]
                        dst = of2[:, b % 2, 0:w7]
                        if eng is nc.scalar:
                            nc.scalar.copy(dst, ps[:, 0:w7])
                        else:
                            eng.tensor_copy(dst, ps[:, 0:w7])
                        if b % 2 == 1:
                            # (m, m+4) pair fold: one 64-partition add
                            # E[16m+i, bp, v] = of2[16m+i, bp, v+4]
                            #                 + of2[16(m+4)+i, bp, v]
                            of2f = of2[:]
                            pstr = of2f.ap[0][0]
                            in0 = bass.AP(tensor=of2f.tensor,
                                          offset=of2f.offset + 4,
                                          ap=[[pstr, 64], [OFW2, 2], [1, w3]])
                            in1 = bass.AP(tensor=of2f.tensor,
                                          offset=of2f.offset + 64 * pstr,
                                          ap=[[pstr, 64], [OFW2, 2], [1, w3]])
                            nc.vector.tensor_tensor(
                                E[:, b - 1:b + 1, 0:w3], in0, in1, ALU.add)
                    # bounce the folded taps to DRAM (HWDGE, one DMA)
                    nc.sync.dma_start(out=dbuf[:, :, 0:w3], in_=E[:, :, 0:w3])
                    # shifted gather: t4[8p+b, m, u] = E[16m+p, b, u+3-m]
                    dbf = dbuf[:]
                    t4 = t4pool.tile([128, 4, 342], BF16, tag="T4",
                                     name=f"t4_{w0}")
                    nc.sync.dma_start(
                        out=t4[:, :, 0:wt],
                        in_=bass.AP(
                            tensor=dbf.tensor, offset=dbf.offset + 3,
                            ap=[[BPC * W3A, 16], [W3A, 8],
                                [16 * BPC * W3A - 1, 4], [1, wt]]),
                    )
                    # fold the 4 tap groups: short bf16 tree, f32 root + bias
                    q0 = qpool.tile([128, 342], BF16, tag="q0",
                                    name=f"q0_{w0}")
                    q1 = qpool.tile([128, 342], BF16, tag="q1",
                                    name=f"q1_{w0}")
                    nc.vector.tensor_add(q0[:, 0:wt], t4[:, 0, 0:wt],
                                         t4[:, 1, 0:wt])
                    nc.gpsimd.tensor_add(q1[:, 0:wt], t4[:, 2, 0:wt],
                                         t4[:, 3, 0:wt])
                    ya = qpool.tile([128, 342], F32, tag="ya",
                                    name=f"ya_{w0}")
                    nc.vector.tensor_add(ya[:, 0:wt], q0[:, 0:wt],
                                         q1[:, 0:wt])
                    yb = qpool.tile([128, 342], F32, tag="yb",
                                    name=f"yb_{w0}")
                    nc.scalar.activation(out=yb[:, 0:wt], in_=ya[:, 0:wt],
                                         func=AF.Identity, bias=cpb[:, 0:1],
                                         scale=1.0)
                    nc.sync.dma_start(
                        out=bass.AP(tensor=y_t, offset=(w0 - 16),
                                    ap=[[2048, 16], [16 * 2048, 8], [1, wt]]),
                        in_=yb[:, 0:wt],
                    )
    nc.compile()
    return nc


_NC_CACHE = None


def _get_nc():
    global _NC_CACHE
    if _NC_CACHE is None:
        _NC_CACHE = _build()
    return _NC_CACHE


def _host_prep(inputs):
    import ml_dtypes

    conv_w = np.asarray(inputs["conv_w"], dtype=np.float32)
    conv_b = np.asarray(inputs["conv_b"], dtype=np.float32)
    conv_gate = np.asarray(inputs["conv_gate"], dtype=np.float32)
    conv_scale = np.asarray(inputs["conv_scale"], dtype=np.float32)
    bn_gamma = np.asarray(inputs["bn_gamma"], dtype=np.float32)
    bn_beta = np.asarray(inputs["bn_beta"], dtype=np.float32)
    ct_w = np.asarray(inputs["ct_w"], dtype=np.float32)
    ct_b = np.asarray(inputs["ct_b"], dtype=np.float32)
    ct_gate = np.asarray(inputs["ct_gate"], dtype=np.float32)
    ct_scale = np.asarray(inputs["ct_scale"], dtype=np.float32)

    W1 = conv_w[:, 0, :] * (conv_gate[:, 0, :] + 1.0) * 0.5  # [c, j]
    W1 = W1 * conv_scale[:, None]
    bias1 = conv_scale * conv_b
    # w1t[j0, h, c] = W1[c, j0 + 128h]
    w1t = np.ascontiguousarray(
        W1.T.reshape(2, 128, K).transpose(1, 0, 2)).astype(ml_dtypes.bfloat16)

    vecs = np.stack([bias1, bn_gamma, bn_beta], axis=1)  # [256, 3]
    vecs = np.ascontiguousarray(vecs.reshape(2, 128, 3).transpose(1, 0, 2))

    W2 = ct_w[:, 0, :] * (ct_gate[:, 0, :] + 1.0) * 0.5  # [k, j]
    W2 = W2 * float(ct_scale[0])
    w2 = np.ascontiguousarray(W2.reshape(2, 128, K).transpose(1, 0, 2))
    w2fold = W2.reshape(K, 16, 16).sum(axis=1)  # [k, p]
    w2fold = np.ascontiguousarray(w2fold.reshape(2, 128, 16).transpose(1, 0, 2))
    cb16 = np.full(16, float(ct_scale[0]) * float(ct_b[0]), dtype=np.float32)

    return {
        "w1t": w1t,
        "vecs": vecs.astype(np.float32),
        "w2": w2.astype(np.float32),
        "w2fold": w2fold.astype(np.float32),
        "cb16": cb16,
    }


def kernel(**inputs) -> np.ndarray:
    import ml_dtypes

    x = np.asarray(inputs["x"], dtype=np.float32)  # [64, 1, 32768]
    shared = _host_prep(inputs)
    nc = _get_nc()

    in_maps = []
    for c in range(N_CORES):
        shard = x[BPC * c:BPC * (c + 1), 0, :]  # [8, T]
        xpad = np.zeros((BPC, XP), dtype=np.float32)
        xpad[:, K:K + T] = shard
        # phase layout: x_ph[b, p, n] = x_pad[b, 16n + p]
        xph = np.ascontiguousarray(
            xpad.reshape(BPC, PW, 16).transpose(0, 2, 1)).astype(ml_dtypes.bfloat16)
        m = dict(shared)
        m["x_ph"] = xph
        in_maps.append(m)

    res = run_bass_kernel_spmd(nc, in_maps, core_ids=list(range(N_CORES)))
    outs = []
    for c in range(N_CORES):
        yph = res.results[c]["y"].reshape(BPC, 16, 2048)  # [b, p, u]
        outs.append(yph.transpose(0, 2, 1).reshape(BPC, 1, T))
    return np.concatenate(outs, axis=0).astype(np.float32)
